# revision 2
# baseline (speedup 1.0000x reference)
"""Trainium2 Bass kernel for nn_DEINA_JBF (stacked Koopman encoder/scan/decode).

Strategy: expert-parallel over the K=64 aux nets (8 nets per core, both the
x- and u-encoder stacks).  Per core:
  - transposed-MLP encoders (features on partitions) in fp32r at full PE rate
  - BN folded into L1 weights on host (stats over the full batch, shared)
  - psi(0) folded into the u-L3 bias; encoder scale folded into W3
  - the 63-step linear scan is re-expressed as a causal matmul over
    (time x net) with host-built lambda-power weight matrices (fp16)
  - y / y_pred decode with C folded with the complex recombination
Host sums the 8 per-core partial outputs (sum over nets is linear).
"""
import os
import numpy as np

import concourse.bacc as bacc
import concourse.mybir as mybir
import concourse.tile as tile
from concourse.bass_utils import run_bass_kernel_spmd

F32 = mybir.dt.float32
F32R = mybir.dt.float32r
F16 = mybir.dt.float16
ACT_COPY = mybir.ActivationFunctionType.Copy
ACT_LRELU = mybir.ActivationFunctionType.Lrelu

N_CORES = 8
NETS = 8          # nets per core
NB, T, DX, DU, H, L, K = 64, 64, 16, 8, 256, 128, 64
M = NB * T        # 4096, m = b*64 + t (u padded at t=63)
PAIRS = NB // 2   # 32 batch pairs
SCOLS = 2 * (T - 1)  # 126 = (b2, s)

# L1 PSUM drains routed to DVE for this many u-nets (ACT/DVE load balance)
DVE_L1U_NETS = 7


def _build():
    nc = bacc.Bacc(None, target_bir_lowering=False, debug=False)

    xnT = nc.dram_tensor("xnT", [DX + 1, M], F32R, kind="ExternalInput")
    unT = nc.dram_tensor("unT", [DU + 1, M], F32R, kind="ExternalInput")
    xw1 = nc.dram_tensor("xw1", [NETS, DX + 1, H], F32R, kind="ExternalInput")
    uw1 = nc.dram_tensor("uw1", [NETS, DU + 1, H], F32R, kind="ExternalInput")
    xw2 = nc.dram_tensor("xw2", [NETS, 128, 512], F32R, kind="ExternalInput")
    uw2 = nc.dram_tensor("uw2", [NETS, 128, 512], F32R, kind="ExternalInput")
    xw3 = nc.dram_tensor("xw3", [NETS, 128, 256], F32R, kind="ExternalInput")
    uw3 = nc.dram_tensor("uw3", [NETS, 128, 256], F32R, kind="ExternalInput")
    xb2 = nc.dram_tensor("xb2", [128, 2 * NETS], F32, kind="ExternalInput")
    ub2 = nc.dram_tensor("ub2", [128, 2 * NETS], F32, kind="ExternalInput")
    ub3 = nc.dram_tensor("ub3", [128, NETS], F32, kind="ExternalInput")
    wr = nc.dram_tensor("wr", [128, NETS * SCOLS], F16, kind="ExternalInput")
    wi = nc.dram_tensor("wi", [128, NETS * SCOLS], F16, kind="ExternalInput")
    c1 = nc.dram_tensor("c1", [128, DX], F16, kind="ExternalInput")
    c2 = nc.dram_tensor("c2", [128, DX], F16, kind="ExternalInput")
    cy = nc.dram_tensor("cy", [128, DX], F32R, kind="ExternalInput")
    ident = nc.dram_tensor("ident", [DX, DX], F32R, kind="ExternalInput")

    y_pad = nc.dram_tensor("y_pad", [M, DX], F32, kind="ExternalOutput")
    ypred = nc.dram_tensor("ypred", [NB * (T - 1), DX], F32, kind="ExternalOutput")

    with tile.TileContext(nc) as tc:
        with (
            tc.tile_pool(name="dram", bufs=1, space="DRAM") as dram,
            tc.tile_pool(name="persist", bufs=1) as persist,
            tc.tile_pool(name="wpool", bufs=2) as wpool,
            tc.tile_pool(name="hpool", bufs=2) as hpool,
            tc.tile_pool(name="stpool", bufs=1) as stpool,
        ):
            psiu_dram = dram.tile([128, NETS * M], F16)

            xnT_sb = persist.tile([DX + 1, M], F32R)
            unT_sb = persist.tile([DU + 1, M], F32R)
            nc.sync.dma_start(xnT_sb[:], xnT[:])
            nc.sync.dma_start(unT_sb[:], unT[:])
            xb2_sb = persist.tile([128, 2 * NETS], F32)
            ub2_sb = persist.tile([128, 2 * NETS], F32)
            ub3_sb = persist.tile([128, NETS], F32)
            nc.sync.dma_start(xb2_sb[:], xb2[:])
            nc.sync.dma_start(ub2_sb[:], ub2[:])
            nc.sync.dma_start(ub3_sb[:], ub3[:])
            wr_sb = persist.tile([128, NETS * SCOLS], F16)
            wi_sb = persist.tile([128, NETS * SCOLS], F16)
            nc.sync.dma_start(wr_sb[:], wr[:])
            nc.sync.dma_start(wi_sb[:], wi[:])
            c1_sb = persist.tile([128, DX], F16)
            c2_sb = persist.tile([128, DX], F16)
            cy_sb = persist.tile([128, DX], F32R)
            ident_sb = persist.tile([DX, DX], F32R)
            nc.sync.dma_start(c1_sb[:], c1[:])
            nc.sync.dma_start(c2_sb[:], c2[:])
            nc.sync.dma_start(cy_sb[:], cy[:])
            nc.sync.dma_start(ident_sb[:], ident[:])

            phiacc = persist.tile([128, M], F32R)
            vt = persist.tile([128, NETS * M], F16)  # transposed psi ((b,t) major)

            with tc.tile_pool(name="eps", bufs=4, space="PSUM") as eps:

                def layer12(w1sb, w2sb, b2col, p, rhsT, din, jj, dve_l1):
                    """L1+L2 for one M-pair; returns (h2s0, h2s1)."""
                    h1a = eps.tile([128, 1024], F32, tag="eps", name="h1a")
                    h1b = eps.tile([128, 1024], F32, tag="eps", name="h1b")
                    for f, hp in ((0, h1a), (1, h1b)):
                        for hh in (0, 1):
                            nc.tensor.matmul(
                                hp[:, hh * 512:hh * 512 + 512],
                                w1sb[:din + 1, f * 128:f * 128 + 128],
                                rhsT[:din + 1, p * 1024 + hh * 512:p * 1024 + hh * 512 + 512],
                                start=True, stop=True,
                            )
                    h1s0 = hpool.tile([128, 1024], F32R, tag="h1s0", name="h1s0")
                    h1s1 = hpool.tile([128, 1024], F32R, tag="h1s1", name="h1s1")
                    for hp, hs in ((h1a, h1s0), (h1b, h1s1)):
                        if dve_l1:
                            tmp = hpool.tile([128, 1024], F32R, tag="ltmp", name="ltmp")
                            nc.vector.tensor_scalar_mul(tmp[:], hp[:], 0.01)
                            nc.vector.tensor_max(hs[:], tmp[:], hp[:])
                        else:
                            nc.scalar.activation(hs[:], hp[:], ACT_LRELU, alpha=0.01)
                    h2a = eps.tile([128, 1024], F32, tag="eps", name="h2a")
                    h2b = eps.tile([128, 1024], F32, tag="eps", name="h2b")
                    for f, hp2 in ((0, h2a), (1, h2b)):
                        for hh in (0, 1):
                            for g, hs in ((0, h1s0), (1, h1s1)):
                                nc.tensor.matmul(
                                    hp2[:, hh * 512:hh * 512 + 512],
                                    w2sb[:, g * 256 + f * 128:g * 256 + f * 128 + 128],
                                    hs[:, hh * 512:hh * 512 + 512],
                                    start=(g == 0), stop=(g == 1),
                                )
                    h2s0 = hpool.tile([128, 1024], F32R, tag="h2s0", name="h2s0")
                    h2s1 = hpool.tile([128, 1024], F32R, tag="h2s1", name="h2s1")
                    for f, (hp2, hs2) in ((0, (h2a, h2s0)), (1, (h2b, h2s1))):
                        nc.scalar.activation(
                            hs2[:], hp2[:], ACT_LRELU, alpha=0.01,
                            bias=b2col[:, 2 * jj + f:2 * jj + f + 1],
                        )
                    return h2s0, h2s1

                def layer3(w3sb, p, h2s0, h2s1):
                    l3 = eps.tile([128, 1024], F32, tag="eps", name="l3")
                    for hh in (0, 1):
                        for g, hs2 in ((0, h2s0), (1, h2s1)):
                            nc.tensor.matmul(
                                l3[:, hh * 512:hh * 512 + 512],
                                w3sb[:, g * 128:g * 128 + 128],
                                hs2[:, hh * 512:hh * 512 + 512],
                                start=(g == 0), stop=(g == 1),
                            )
                    return l3

                for jj in range(NETS):
                    stage = stpool.tile([128, M], F16, tag="stage", name="stage")

                    # ---- x-net jj ----
                    xw1sb = wpool.tile([DX + 1, H], F32R, tag="xw1", name="xw1sb")
                    xw2sb = wpool.tile([128, 512], F32R, tag="xw2", name="xw2sb")
                    xw3sb = wpool.tile([128, 256], F32R, tag="xw3", name="xw3sb")
                    nc.sync.dma_start(xw1sb[:], xw1[jj])
                    nc.sync.dma_start(xw2sb[:], xw2[jj])
                    nc.sync.dma_start(xw3sb[:], xw3[jj])
                    for p in range(4):
                        h2s0, h2s1 = layer12(
                            xw1sb, xw2sb, xb2_sb, p, xnT_sb, DX, jj, False)
                        l3 = layer3(xw3sb, p, h2s0, h2s1)
                        sl = phiacc[:, p * 1024:(p + 1) * 1024]
                        if jj == 0:
                            nc.vector.tensor_scalar_add(sl, l3[:], 0.0)
                        else:
                            nc.vector.tensor_add(sl, l3[:], sl)
                        # phi0 (t=0 cols) -> stage t=63 slots, fp16
                        l3v = l3[:].rearrange("q (b t) -> q b t", t=T)[:, :, 0]
                        stv = stage[:, p * 1024:(p + 1) * 1024].rearrange(
                            "q (b t) -> q b t", t=T)[:, :, T - 1]
                        nc.scalar.activation(stv, l3v, ACT_COPY)

                    # ---- u-net jj ----
                    uw1sb = wpool.tile([DU + 1, H], F32R, tag="uw1", name="uw1sb")
                    uw2sb = wpool.tile([128, 512], F32R, tag="uw2", name="uw2sb")
                    uw3sb = wpool.tile([128, 256], F32R, tag="uw3", name="uw3sb")
                    nc.sync.dma_start(uw1sb[:], uw1[jj])
                    nc.sync.dma_start(uw2sb[:], uw2[jj])
                    nc.sync.dma_start(uw3sb[:], uw3[jj])
                    for p in range(4):
                        h2s0, h2s1 = layer12(
                            uw1sb, uw2sb, ub2_sb, p, unT_sb, DU, jj,
                            jj < DVE_L1U_NETS)
                        l3 = layer3(uw3sb, p, h2s0, h2s1)
                        # psi - psi0 -> stage (t<63 cols), fp16
                        l3v = l3[:].rearrange("q (b t) -> q b t", t=T)[:, :, 0:T - 1]
                        stv = stage[:, p * 1024:(p + 1) * 1024].rearrange(
                            "q (b t) -> q b t", t=T)[:, :, 0:T - 1]
                        nc.vector.tensor_scalar_add(
                            stv, l3v, ub3_sb[:, jj:jj + 1])

                    # stage -> DRAM -> transposed vt (single XBAR DMA each)
                    nc.sync.dma_start(
                        psiu_dram[:, jj * M:(jj + 1) * M], stage[:])
                    vt3d = vt[:, jj * M:(jj + 1) * M].rearrange(
                        "q (f c) -> q f c", c=128)
                    nc.sync.dma_start(
                        vt3d, psiu_dram[:, jj * M:(jj + 1) * M], transpose=True)

            with (
                tc.tile_pool(name="yps", bufs=1, space="PSUM") as yps,
                tc.tile_pool(name="ytps", bufs=2, space="PSUM") as ytps,
                tc.tile_pool(name="sps", bufs=1, space="PSUM") as sps,
                tc.tile_pool(name="dps", bufs=1, space="PSUM") as dps,
                tc.tile_pool(name="ytps2", bufs=2, space="PSUM") as ytps2,
                tc.tile_pool(name="spool", bufs=2) as spool,
            ):
                # ---- y path: decode phiacc ----
                for q in range(8):
                    yq = yps.tile([DX, 512], F32, tag="yq", name="yq")
                    nc.tensor.matmul(
                        yq[:], cy_sb[:], phiacc[:, q * 512:(q + 1) * 512],
                        start=True, stop=True)
                    yqs = spool.tile([DX, 512], F32R, tag="yqs", name="yqs")
                    nc.scalar.activation(yqs[:], yq[:], ACT_COPY)
                    for hh in range(4):
                        ytp = ytps.tile([128, DX], F32R, tag="ytp", name="ytp")
                        nc.tensor.transpose(
                            ytp[:], yqs[:, hh * 128:(hh + 1) * 128],
                            ident_sb[:])
                        yts = spool.tile([128, DX], F32, tag="yts", name="yts")
                        nc.vector.tensor_copy(yts[:], ytp[:])
                        r0 = q * 512 + hh * 128
                        nc.sync.dma_start(y_pad[r0:r0 + 128, :], yts[:])

                # ---- scan + y_pred ----
                for pair in range(PAIRS):
                    p1 = sps.tile([128, SCOLS], F32, tag="p1", name="p1")
                    p2 = sps.tile([128, SCOLS], F32, tag="p2", name="p2")
                    for jj in range(NETS):
                        vtile = vt[:, jj * M + pair * 128:jj * M + (pair + 1) * 128]
                        nc.tensor.matmul(
                            p1[:], vtile, wr_sb[:, jj * SCOLS:(jj + 1) * SCOLS],
                            start=(jj == 0), stop=(jj == NETS - 1))
                    for jj in range(NETS):
                        vtile = vt[:, jj * M + pair * 128:jj * M + (pair + 1) * 128]
                        nc.tensor.matmul(
                            p2[:], vtile, wi_sb[:, jj * SCOLS:(jj + 1) * SCOLS],
                            start=(jj == 0), stop=(jj == NETS - 1))
                    p1s = spool.tile([128, SCOLS], F16, tag="p1s", name="p1s")
                    p2s = spool.tile([128, SCOLS], F16, tag="p2s", name="p2s")
                    nc.vector.tensor_copy(p1s[:], p1[:])
                    nc.vector.tensor_copy(p2s[:], p2[:])
                    yp = dps.tile([DX, SCOLS], F32, tag="yp", name="yp")
                    nc.tensor.matmul(yp[:], c1_sb[:], p1s[:],
                                     start=True, stop=False)
                    nc.tensor.matmul(yp[:], c2_sb[:], p2s[:],
                                     start=False, stop=True)
                    ypsb = spool.tile([DX, SCOLS], F32R, tag="ypsb", name="ypsb")
                    nc.scalar.activation(ypsb[:], yp[:], ACT_COPY)
                    ytp2 = ytps2.tile([SCOLS, DX], F32R, tag="ytp2", name="ytp2")
                    nc.tensor.transpose(ytp2[:], ypsb[:], ident_sb[:])
                    yts2 = spool.tile([SCOLS, DX], F32, tag="yts2", name="yts2")
                    nc.vector.tensor_copy(yts2[:], ytp2[:])
                    nc.sync.dma_start(
                        ypred[pair * SCOLS:(pair + 1) * SCOLS, :], yts2[:])

    nc.compile()
    return nc


_NC = None


def _get_nc():
    global _NC
    if _NC is None:
        _NC = _build()
    return _NC


def _leaky(x):
    return np.where(x > 0, x, 0.01 * x)


def _prep(xs, us, x_gamma, x_beta, xW1, xb1, xW2, xb2, xW3, x_scale,
          u_gamma, u_beta, uW1, ub1, uW2, ub2, uW3, u_scale, reL, imL, C_W):
    """Host-side fold + shard. Returns per-core input maps."""
    f32 = np.float32
    xs, us = np.asarray(xs, f32), np.asarray(us, f32)
    perm = np.concatenate([np.arange(K) * 2, np.arange(K) * 2 + 1])

    # BN (training stats, full batch — identical on every core)
    xf = xs.reshape(M, DX)
    mu, var = xf.mean(0), xf.var(0)
    xn = (xf - mu) / np.sqrt(var + 1e-5)
    uf = us.reshape(NB * (T - 1), DU)
    muu, varu = uf.mean(0), uf.var(0)
    un = (uf - muu) / np.sqrt(varu + 1e-5)
    un_pad = np.zeros((M, DU), f32)
    un_pad.reshape(NB, T, DU)[:, :T - 1] = un.reshape(NB, T - 1, DU)

    xnT = np.concatenate([xn.T, np.ones((1, M), f32)], 0)
    unT = np.concatenate([un_pad.T, np.ones((1, M), f32)], 0)

    # fold BN affine into L1; scale into L3 (+ (c,k) feature permutation)
    xW1f = np.asarray(x_gamma, f32)[:, :, None] * np.asarray(xW1, f32)
    xb1f = np.einsum("kd,kdh->kh", np.asarray(x_beta, f32),
                     np.asarray(xW1, f32)) + np.asarray(xb1, f32)
    uW1f = np.asarray(u_gamma, f32)[:, :, None] * np.asarray(uW1, f32)
    ub1f = np.einsum("kd,kdh->kh", np.asarray(u_beta, f32),
                     np.asarray(uW1, f32)) + np.asarray(ub1, f32)
    xW3f = (np.asarray(xW3, f32) * np.asarray(x_scale, f32)[:, None, :])[:, :, perm]
    uW3f = (np.asarray(uW3, f32) * np.asarray(u_scale, f32)[:, None, :])[:, :, perm]

    # psi(0): encoder of the zero input (BN of zeros -> 0 -> beta path)
    h0 = _leaky(ub1f)                                   # [K, H]
    h0 = _leaky(np.einsum("kh,khg->kg", h0, np.asarray(uW2, f32))
                + np.asarray(ub2, f32))
    psi0 = np.einsum("kh,khl->kl", h0, uW3f)            # already permuted

    # lambda-power scan weights
    lam = np.asarray(reL, f32) + 1j * np.asarray(imL, f32)  # [K]
    s_idx = np.arange(T - 1)
    pw = np.zeros((K, T, T - 1), np.complex64)
    for t in range(T - 1):
        e = s_idx + 1 - t
        pw[:, t, :] = np.where(s_idx >= t, lam[:, None] ** e[None, :], 0)
    pw[:, T - 1, :] = lam[:, None] ** (s_idx + 1)[None, :]

    CW = np.asarray(C_W, f32)                            # [DX, L]
    Cre, Cim = CW[:, 0::2], CW[:, 1::2]                  # [DX, K]
    c1 = np.concatenate([Cre.T, Cim.T], 0).astype(np.float16)   # [128, DX]
    c2 = np.concatenate([Cim.T, -Cre.T], 0).astype(np.float16)
    cy = CW[:, perm].T.copy()                            # [128, DX]
    ident = np.eye(DX, dtype=f32)

    maps = []
    for c in range(N_CORES):
        js = slice(c * NETS, (c + 1) * NETS)
        xw2_h = np.ascontiguousarray(
            np.asarray(xW2, f32)[js].reshape(NETS, 2, 128, 2, 128)
            .transpose(0, 2, 1, 3, 4).reshape(NETS, 128, 512))
        uw2_h = np.ascontiguousarray(
            np.asarray(uW2, f32)[js].reshape(NETS, 2, 128, 2, 128)
            .transpose(0, 2, 1, 3, 4).reshape(NETS, 128, 512))
        xw3_h = np.ascontiguousarray(
            xW3f[js].reshape(NETS, 2, 128, 128)
            .transpose(0, 2, 1, 3).reshape(NETS, 128, 256))
        uw3_h = np.ascontiguousarray(
            uW3f[js].reshape(NETS, 2, 128, 128)
            .transpose(0, 2, 1, 3).reshape(NETS, 128, 256))
        xw1_h = np.concatenate([xW1f[js], xb1f[js][:, None, :]], 1)
        uw1_h = np.concatenate([uW1f[js], ub1f[js][:, None, :]], 1)
        xb2_h = np.ascontiguousarray(
            np.asarray(xb2, f32)[js].reshape(NETS, 2, 128)
            .transpose(2, 0, 1).reshape(128, 2 * NETS))
        ub2_h = np.ascontiguousarray(
            np.asarray(ub2, f32)[js].reshape(NETS, 2, 128)
            .transpose(2, 0, 1).reshape(128, 2 * NETS))
        ub3_h = np.ascontiguousarray((-psi0[js]).T)      # [128, NETS]

        wr_h = np.zeros((128, NETS * SCOLS), np.float16)
        wi_h = np.zeros((128, NETS * SCOLS), np.float16)
        for jl in range(NETS):
            pj = pw[c * NETS + jl]                       # [T, T-1]
            for b2 in range(2):
                r0, c0 = b2 * T, jl * SCOLS + b2 * (T - 1)
                wr_h[r0:r0 + T, c0:c0 + T - 1] = pj.real
                wi_h[r0:r0 + T, c0:c0 + T - 1] = pj.imag

        maps.append({
            "xnT": xnT, "unT": unT,
            "xw1": np.ascontiguousarray(xw1_h), "uw1": np.ascontiguousarray(uw1_h),
            "xw2": xw2_h, "uw2": uw2_h, "xw3": xw3_h, "uw3": uw3_h,
            "xb2": xb2_h, "ub2": ub2_h, "ub3": ub3_h,
            "wr": wr_h, "wi": wi_h, "c1": c1, "c2": c2, "cy": cy,
            "ident": ident,
        })
    return maps


def _run(inputs, trace=False):
    nc = _get_nc()
    maps = _prep(**inputs)
    res = run_bass_kernel_spmd(nc, maps, list(range(N_CORES)), trace=trace)
    y = np.zeros((M, DX), np.float32)
    yp = np.zeros((NB * (T - 1), DX), np.float32)
    for c in range(N_CORES):
        y += res.results[c]["y_pad"]
        yp += res.results[c]["ypred"]
    y = y.reshape(NB, T, DX)[:, 1:, :]
    yp = yp.reshape(NB, T - 1, DX)
    return (y, yp), res


def kernel(**inputs):
    (y, yp), _ = _run(inputs, trace=bool(os.environ.get("BASS_KERNEL_TRACE")))
    return y, yp
